# revision 3
# baseline (speedup 1.0000x reference)
"""MixHop GNN kernel v2 for one TRN2 chip (8 NeuronCores), Bass/Tile.

Math (matches the reference):
    P = D^-1/2 (A+I) D^-1/2 with deg over targets (+self)
    h1 = P x ; h2 = P h1
    out = relu([x@W0+b0, h1@W1+b1, h2@W2+b2]) @ Wout + bout

Design vs v1 baseline:
  - Per-edge norm w = dinv[src]*dinv[tgt] is folded into the one-hot
    segment matrix S (tensor_scalar is_equal + mult, one DVE op per
    128-edge block) -> no pre/post scaling passes, no z staging.
  - Self loops are ordinary edges (w = dinv^2) -> no special-casing.
  - Node windows (128 targets) are assigned to (core, slot) by balanced
    snake assignment over edge counts -> minimal SPMD padding. The
    gather table (x_tab / h1full) lives in this permuted "position"
    space, so hop1 and hop2 share one int16 index stream.
  - Gathers are issued as long streams chunked into MAXB-block calls
    (optionally multi-packet, optionally spread over SWDGE queues).
  - hop1 emits node-major h1 (staged for AllGather); hop2 emits
    feature-major h2.T directly (flipped matmul) feeding the dense head
    without extra transposes. x arrives pre-transposed per core.
"""
import numpy as np
import ml_dtypes

N = 50000
F = 128
OUT = 64
PER_HOP = 64
NCORE = 8
WIN = 128
NSLOT = 49                  # slots per core (8*49=392 >= 391 windows)
NPCP = NSLOT * WIN          # 6272 padded nodes per core
POS_N = NCORE * NPCP        # 50176 positions
NWTOT = (N + WIN - 1) // WIN  # 391 real windows
SPLIT = 32768               # int16 index limit
PAD_TL = 300.0
# chunked AllGather: early chunks overlap later compute; the last chunk
# is small so its serial exposure is minimal
CH_SLOTS = (25, 16, 8)
CH_BASE = (0, NCORE * 25 * WIN, NCORE * 41 * WIN)

# tuned via micro-benchmarks: 4096-idx multi-packet calls spread over
# all 4 SWDGE queues reach ~3.6ns/descriptor vs ~8.7 for the default path
MAXB = 32                   # blocks (128 idxs each) per dma_gather call
SINGLE_PACKET = False
NQUEUES = 4


def _assign_windows(cnt):
    """Balanced snake assignment of windows to (core, slot)."""
    order = list(np.argsort(-cnt[:NWTOT], kind="stable")) + [NWTOT]
    w2core = np.zeros(NWTOT + 1, dtype=np.int64)
    w2slot = np.zeros(NWTOT + 1, dtype=np.int64)
    for band in range(NSLOT):
        grp = order[band * NCORE:(band + 1) * NCORE]
        cores = range(NCORE) if band % 2 == 0 else range(NCORE - 1, -1, -1)
        for c, w in zip(cores, grp):
            w2core[w] = c
            w2slot[w] = band
    return w2core, w2slot


def _preprocess(edge_index):
    row = np.asarray(edge_index[0], dtype=np.int64)
    col = np.asarray(edge_index[1], dtype=np.int64)
    deg = (np.bincount(col, minlength=N) + 1.0).astype(np.float64)
    dinv = 1.0 / np.sqrt(deg)

    nodes = np.arange(N)
    wof = nodes // WIN
    cnt = np.bincount(col // WIN, minlength=NWTOT + 1)
    w2core, w2slot = _assign_windows(cnt)
    # chunk-major position space (matches the chunked AllGather layout)
    wc, ws, lane = w2core[wof], w2slot[wof], nodes % WIN
    cum = np.cumsum((0,) + CH_SLOTS)
    chix = np.searchsorted(cum, ws, side="right") - 1
    pos = (np.asarray(CH_BASE)[chix] +
           wc * np.asarray(CH_SLOTS)[chix] * WIN +
           (ws - cum[chix]) * WIN + lane)
    # core-major position space (device-local slot order, used for
    # x_own/dinv2 tables and output reassembly)
    posc = wc * NPCP + ws * WIN + lane

    # self loops are handled on-device via diagonal blocks; only real
    # edges go through the gather path
    src_pos = pos[row]
    tgt = col
    wgt = (dinv[row] * dinv[col]).astype(np.float32)
    tw = tgt // WIN
    ecore = w2core[tw]
    eslot = w2slot[tw]
    etl = (tgt % WIN).astype(np.float32)
    is_lo = src_pos < SPLIT

    # per (core, slot, half) edge lists
    lists = [[[None, None] for _ in range(NSLOT)] for _ in range(NCORE)]
    key = (ecore * NSLOT + eslot) * 2 + (~is_lo)
    order = np.argsort(key, kind="stable")
    ks = key[order]
    bounds = np.searchsorted(ks, np.arange(NCORE * NSLOT * 2 + 1))
    for c in range(NCORE):
        for s in range(NSLOT):
            for h in range(2):
                k = (c * NSLOT + s) * 2 + h
                sel = order[bounds[k]:bounds[k + 1]]
                sp = src_pos[sel] - (SPLIT if h else 0)
                lists[c][s][h] = (sp.astype(np.int64), etl[sel], wgt[sel])

    NBL = [max(int(np.ceil(len(lists[c][s][0][0]) / 128))
               for c in range(NCORE)) for s in range(NSLOT)]
    NBH = [max(int(np.ceil(len(lists[c][s][1][0]) / 128))
               for c in range(NCORE)) for s in range(NSLOT)]

    NBL_TOT, NBH_TOT = sum(NBL), sum(NBH)
    NBT_TOT = NBL_TOT + NBH_TOT

    # per-core padded streams: lo blocks (slot-major) then hi blocks
    per_core = []
    for c in range(NCORE):
        idx = np.zeros(NBT_TOT * 128, dtype=np.int16)
        tl = np.full(NBT_TOT * 128, PAD_TL, dtype=np.float32)
        wv = np.zeros(NBT_TOT * 128, dtype=np.float32)
        off = 0
        for h, nbs in ((0, NBL), (1, NBH)):
            for s in range(NSLOT):
                sp, t, w = lists[c][s][h]
                n = len(sp)
                idx[off:off + n] = sp
                tl[off:off + n] = t
                wv[off:off + n] = w
                off += nbs[s] * 128
        # idx stream -> per-call [16-wrap x8] layout
        calls = _call_sizes(NBL_TOT) + _call_sizes(NBH_TOT)
        icols = sum(nb * 8 for nb in calls)
        idx16 = np.zeros((128, icols), dtype=np.int16)
        io = 0
        bo = 0
        for nb in calls:
            a = idx[bo * 128:(bo + nb) * 128]
            idx16[:, io:io + nb * 8] = np.tile(a.reshape(-1, 16).T, (8, 1))
            io += nb * 8
            bo += nb
        # streamed S blocks: s_all[e, j*128+t] = w_e * (tl_e == t)
        arr = np.zeros((NBT_TOT, 128, 128), dtype=ml_dtypes.bfloat16)
        blk = np.arange(NBT_TOT * 128) // 128
        lane_e = np.arange(NBT_TOT * 128) % 128
        valid = tl < 128
        arr[blk[valid], lane_e[valid], tl[valid].astype(np.int64)] = \
            wv[valid].astype(ml_dtypes.bfloat16)
        s_all = np.ascontiguousarray(
            arr.transpose(1, 0, 2).reshape(128, NBT_TOT * 128))
        # dinv^2 per (lane, slot) for the on-device self-loop diagonal
        d2 = np.zeros(POS_N, dtype=np.float32)
        d2[posc] = (dinv * dinv).astype(np.float32)
        d2c = d2[c * NPCP:(c + 1) * NPCP].reshape(NSLOT, WIN).T
        per_core.append({"idx16": idx16, "s_all": s_all,
                         "dinv2_t": np.ascontiguousarray(d2c)})

    return tuple(NBL), tuple(NBH), per_core, pos, posc


def _call_sizes(nb_tot):
    out = []
    while nb_tot > 0:
        c = min(nb_tot, MAXB)
        out.append(c)
        nb_tot -= c
    return out


def _build(NBL, NBH):
    import concourse.bass as bass  # noqa: F401
    import concourse.bacc as bacc
    import concourse.tile as tile
    import concourse.mybir as mybir

    dt = mybir.dt
    f32 = dt.float32
    bf16 = dt.bfloat16
    AF = mybir.ActivationFunctionType
    ALU = mybir.AluOpType

    NBL_TOT, NBH_TOT = sum(NBL), sum(NBH)
    NBT_TOT = NBL_TOT + NBH_TOT
    calls_lo = _call_sizes(NBL_TOT)
    calls_hi = _call_sizes(NBH_TOT)
    ICOLS = sum(nb * 8 for nb in calls_lo + calls_hi)
    MAXNB = max(calls_lo + calls_hi)

    nc = bacc.Bacc("TRN2", target_bir_lowering=False, debug=False,
                   num_devices=NCORE, num_swdge_queues=NQUEUES)

    x_tab = nc.dram_tensor("x_tab", [POS_N, F], bf16, kind="ExternalInput")
    xT_in = nc.dram_tensor("x_ownT", [128, NPCP], bf16, kind="ExternalInput")
    xo_in = nc.dram_tensor("x_own", [NPCP, F], bf16, kind="ExternalInput")
    d2_in = nc.dram_tensor("dinv2_t", [128, NSLOT], f32,
                           kind="ExternalInput")
    idx_in = nc.dram_tensor("idx16", [128, ICOLS], dt.int16,
                            kind="ExternalInput")
    s_in = nc.dram_tensor("s_all", [128, NBT_TOT * 128], bf16,
                          kind="ExternalInput")
    id_in = nc.dram_tensor("ident", [128, 128], bf16, kind="ExternalInput")
    w0_in = nc.dram_tensor("w0", [F, PER_HOP], bf16, kind="ExternalInput")
    w1_in = nc.dram_tensor("w1", [F, PER_HOP], bf16, kind="ExternalInput")
    w2_in = nc.dram_tensor("w2", [F, PER_HOP], bf16, kind="ExternalInput")
    wo_in = nc.dram_tensor("wout", [3 * PER_HOP, OUT], bf16,
                           kind="ExternalInput")
    b0_in = nc.dram_tensor("b0", [PER_HOP, 1], f32, kind="ExternalInput")
    b1_in = nc.dram_tensor("b1", [PER_HOP, 1], f32, kind="ExternalInput")
    b2_in = nc.dram_tensor("b2", [PER_HOP, 1], f32, kind="ExternalInput")
    bo_in = nc.dram_tensor("bout", [OUT, 1], f32, kind="ExternalInput")
    out_t = nc.dram_tensor("out_t", [OUT, NPCP], f32, kind="ExternalOutput")

    h1own = nc.dram_tensor("h1own", [NPCP, F], bf16)
    h1full = nc.dram_tensor("h1full", [POS_N, F], bf16, addr_space="Shared")

    def ts(s):
        return slice(s * WIN, (s + 1) * WIN)

    with tile.TileContext(nc) as tc:
        with (
            tc.tile_pool(name="persist", bufs=1) as pp,
            tc.tile_pool(name="glo", bufs=3) as glop,
            tc.tile_pool(name="ghi", bufs=3) as ghip,
            tc.tile_pool(name="sblk", bufs=2) as sp_,
            tc.tile_pool(name="work", bufs=4) as wp,
            tc.tile_pool(name="pseg", bufs=2, space="PSUM") as pseg,
            tc.tile_pool(name="ptp", bufs=2, space="PSUM") as ptp,
            tc.tile_pool(name="pcps", bufs=2, space="PSUM") as pcps,
            tc.tile_pool(name="pops", bufs=2, space="PSUM") as pops,
        ):
            # persistent loads
            idx_sb = pp.tile([128, ICOLS], dt.int16)
            nc.sync.dma_start(out=idx_sb[:], in_=idx_in[:])
            x_sb = pp.tile([128, NPCP], bf16)
            nc.sync.dma_start(
                out=x_sb[:].rearrange("p (w f) -> p w f", f=F),
                in_=xo_in.ap().rearrange("(w p) f -> p w f", p=128),
            )
            d2_sb = pp.tile([128, NSLOT], f32)
            nc.sync.dma_start(out=d2_sb[:], in_=d2_in[:])
            ident = pp.tile([128, 128], bf16)
            nc.sync.dma_start(out=ident[:], in_=id_in[:])

            h1_sb = pp.tile([128, NPCP], bf16)
            out_stage = pp.tile([OUT, NPCP], f32)

            def make_streams(src):
                """Two block streams (lo, hi) with lazy chunked gathers."""
                streams = []
                for calls, base, lim, pool, tag in (
                        (calls_lo, 0, SPLIT, glop, "glo"),
                        (calls_hi, SPLIT, POS_N, ghip, "ghi")):
                    st = {"calls": calls, "ci": 0, "tiles": {},
                          "icol0": 0 if base == 0 else
                          sum(nb * 8 for nb in calls_lo),
                          "pool": pool, "tag": tag,
                          "inap": src.ap()[base:lim, :]}
                    streams.append(st)
                return streams

            qctr = [0]

            def issue_call(st, ci):
                calls = st["calls"]
                nb = calls[ci]
                icol = st["icol0"] + sum(c * 8 for c in calls[:ci])
                g = st["pool"].tile([128, MAXNB * F], bf16, tag=st["tag"])
                nc.gpsimd.dma_gather(
                    out_ap=g[:, 0:nb * F].rearrange("p (b f) -> p b f", f=F),
                    in_ap=st["inap"],
                    idxs_ap=idx_sb[:, icol:icol + nb * 8],
                    num_idxs=nb * 128, num_idxs_reg=nb * 128,
                    elem_size=F, queue_num=qctr[0] % NQUEUES,
                    single_packet=SINGLE_PACKET)
                qctr[0] += 1
                st["tiles"][ci] = g

            def get_block(st, j):
                """Return (tile, offset) for stream block j; prefetch ahead
                so several calls are in flight across the SWDGE queues."""
                ci, off = divmod(j, MAXB)
                for c in range(ci, min(ci + 3, len(st["calls"]))):
                    if c not in st["tiles"]:
                        issue_call(st, c)
                # drop old tiles so the pool can recycle
                for k in list(st["tiles"]):
                    if k < ci:
                        del st["tiles"][k]
                return st["tiles"][ci], off

            SCH = 32  # S blocks per streamed chunk
            NSCH = (NBT_TOT + SCH - 1) // SCH

            def make_sstream(tag):
                return {"tiles": {}, "tag": tag}

            def get_sblock(sst, gcol):
                cj, off = divmod(gcol, SCH)
                for c2 in range(cj, min(cj + 2, NSCH)):
                    if c2 not in sst["tiles"]:
                        stile = sp_.tile([128, SCH * 128], bf16,
                                         tag=sst["tag"])
                        lo = c2 * SCH * 128
                        hi = min(NBT_TOT, (c2 + 1) * SCH) * 128
                        nc.sync.dma_start(out=stile[:, 0:hi - lo],
                                          in_=s_in.ap()[:, lo:hi])
                        sst["tiles"][c2] = stile
                for k in list(sst["tiles"]):
                    if k < cj:
                        del sst["tiles"][k]
                return sst["tiles"][cj], off

            def hop(src, flipped, self_sb, consume, post_slot=None):
                streams = make_streams(src)
                sstreams = [make_sstream("slo"), make_sstream("shi")]
                lo_pos = [0]
                hi_pos = [0]
                for s in range(NSLOT):
                    nbl, nbh = NBL[s], NBH[s]
                    nbt = nbl + nbh + 1  # +1 self-loop diagonal block
                    ps = pseg.tile([128, 128], f32, tag="ps")
                    done = 0
                    for h, npos, nblk in ((0, lo_pos, nbl), (1, hi_pos, nbh)):
                        st = streams[h]
                        colbase = 0 if h == 0 else NBL_TOT
                        for k in range(nblk):
                            j = npos[0] + k
                            g, off = get_block(st, j)
                            stile, soff = get_sblock(sstreams[h], colbase + j)
                            sb = stile[:, soff * 128:(soff + 1) * 128]
                            rhs = g[:, off * F:(off + 1) * F]
                            if flipped:
                                nc.tensor.matmul(
                                    out=ps[:], lhsT=rhs, rhs=sb,
                                    start=(done == 0), stop=False)
                            else:
                                nc.tensor.matmul(
                                    out=ps[:], lhsT=sb, rhs=rhs,
                                    start=(done == 0), stop=False)
                            done += 1
                        npos[0] += nblk
                    # self loop: diag(dinv2) block, local data
                    sd = sp_.tile([128, 128], bf16, tag="sd")
                    nc.scalar.activation(out=sd[:], in_=ident[:],
                                         func=AF.Identity,
                                         scale=d2_sb[:, s:s + 1])
                    if flipped:
                        nc.tensor.matmul(out=ps[:], lhsT=self_sb[:, ts(s)],
                                         rhs=sd[:], start=(done == 0),
                                         stop=True)
                    else:
                        nc.tensor.matmul(out=ps[:], lhsT=sd[:],
                                         rhs=self_sb[:, ts(s)],
                                         start=(done == 0), stop=True)
                    consume(s, ps)
                    if post_slot is not None:
                        post_slot(s)

            # ---- hop 1: node-major h1, chunked stage + AllGather ----
            CH_CUM = [0]
            for n_ in CH_SLOTS:
                CH_CUM.append(CH_CUM[-1] + n_)

            def stage_chunk(ch):
                lo = CH_CUM[ch] * WIN
                hi = CH_CUM[ch + 1] * WIN
                flo = CH_BASE[ch]
                fhi = flo + NCORE * CH_SLOTS[ch] * WIN
                nc.sync.dma_start(
                    out=h1own.ap()[lo:hi, :].rearrange(
                        "(w p) f -> p w f", p=128),
                    in_=h1_sb[:, lo:hi].rearrange("p (w f) -> p w f", f=F),
                )
                nc.gpsimd.collective_compute(
                    "AllGather", ALU.bypass,
                    replica_groups=[list(range(NCORE))],
                    ins=[h1own.ap()[lo:hi, :]],
                    outs=[h1full.ap()[flo:fhi, :]])

            def consume1(s, ps):
                nc.scalar.activation(out=h1_sb[:, ts(s)], in_=ps[:],
                                     func=AF.Identity)

            def post1(s):
                for ch in range(len(CH_SLOTS)):
                    if s == CH_CUM[ch + 1] - 1:
                        stage_chunk(ch)

            hop(x_tab, False, x_sb, consume1, post1)

            # head-only inputs load after hop 1 is queued (off the ramp)
            xT_sb = pp.tile([128, NPCP], bf16)
            nc.sync.dma_start(out=xT_sb[:], in_=xT_in[:])
            wk_sb = []
            for k, win_ in enumerate((w0_in, w1_in, w2_in)):
                t = pp.tile([F, PER_HOP], bf16, tag=f"w{k}")
                nc.sync.dma_start(out=t[:], in_=win_[:])
                wk_sb.append(t)
            wo_sb = []
            for k in range(3):
                t = pp.tile([PER_HOP, OUT], bf16, tag=f"wo{k}")
                nc.sync.dma_start(
                    out=t[:], in_=wo_in.ap()[k * PER_HOP:(k + 1) * PER_HOP, :])
                wo_sb.append(t)
            b_sb = []
            for k, bin_ in enumerate((b0_in, b1_in, b2_in)):
                t = pp.tile([PER_HOP, 1], f32, tag=f"b{k}")
                nc.sync.dma_start(out=t[:], in_=bin_[:])
                b_sb.append(t)
            bo_sb = pp.tile([OUT, 1], f32)
            nc.sync.dma_start(out=bo_sb[:], in_=bo_in[:])

            # x/h1 head terms are independent of the collective -> they
            # execute in its shadow
            r0_sb = pp.tile([PER_HOP, NPCP], bf16)
            r1_sb = pp.tile([PER_HOP, NPCP], bf16)
            r01_sb = [r0_sb, r1_sb]
            for s in range(NSLOT):
                tp = ptp.tile([128, 128], bf16, tag="tp")
                nc.tensor.transpose(out=tp[:], in_=h1_sb[:, ts(s)],
                                    identity=ident[:])
                h1T = wp.tile([128, 128], bf16, tag="h1T")
                nc.scalar.activation(out=h1T[:], in_=tp[:],
                                     func=AF.Identity)
                for k, rhs in ((0, xT_sb[:, ts(s)]), (1, h1T[:])):
                    cps = pcps.tile([PER_HOP, 128], f32, tag="cps")
                    nc.tensor.matmul(out=cps[:], lhsT=wk_sb[k][:],
                                     rhs=rhs, start=True, stop=True)
                    nc.scalar.activation(out=r01_sb[k][:, ts(s)], in_=cps[:],
                                         func=AF.Relu, bias=b_sb[k][:])

            # ---- hop 2 (flipped: h2.T) fused with the rest of the head ----
            def consume2(s, ps2):
                h2T = wp.tile([128, 128], bf16, tag="h2T")
                nc.scalar.activation(out=h2T[:], in_=ps2[:], func=AF.Identity)
                cps = pcps.tile([PER_HOP, 128], f32, tag="cps")
                nc.tensor.matmul(out=cps[:], lhsT=wk_sb[2][:],
                                 rhs=h2T[:], start=True, stop=True)
                r2 = wp.tile([PER_HOP, 128], bf16, tag="r2")
                nc.scalar.activation(out=r2[:], in_=cps[:], func=AF.Relu,
                                     bias=b_sb[2][:])
                relus = (r01_sb[0][:, ts(s)], r01_sb[1][:, ts(s)], r2[:])
                ops = pops.tile([OUT, 128], f32, tag="ops")
                for k in range(3):
                    nc.tensor.matmul(out=ops[:], lhsT=wo_sb[k][:],
                                     rhs=relus[k],
                                     start=(k == 0), stop=(k == 2))
                nc.scalar.activation(out=out_stage[:, ts(s)], in_=ops[:],
                                     func=AF.Identity, bias=bo_sb[:])

            hop(h1full, True, h1_sb, consume2)
            nc.sync.dma_start(out=out_t[:], in_=out_stage[:])

    nc.compile()
    return nc


_CACHE = {}


def _get_nc(NBL, NBH):
    key = (NBL, NBH)
    if key not in _CACHE:
        _CACHE[key] = _build(NBL, NBH)
    return _CACHE[key]


def make_in_maps(x, pc, pos, W0, b0, W1, b1, W2, b2, Wout, bout):
    x = np.asarray(x, dtype=np.float32)
    xb = x.astype(ml_dtypes.bfloat16)
    x_tab = np.zeros((POS_N, F), dtype=ml_dtypes.bfloat16)
    x_tab[pos] = xb
    common = {
        "x_tab": x_tab,
        "ident": np.eye(128, dtype=np.float32).astype(ml_dtypes.bfloat16),
        "w0": np.asarray(W0, np.float32).astype(ml_dtypes.bfloat16),
        "w1": np.asarray(W1, np.float32).astype(ml_dtypes.bfloat16),
        "w2": np.asarray(W2, np.float32).astype(ml_dtypes.bfloat16),
        "wout": np.asarray(Wout, np.float32).astype(ml_dtypes.bfloat16),
        "b0": np.asarray(b0, np.float32).reshape(PER_HOP, 1),
        "b1": np.asarray(b1, np.float32).reshape(PER_HOP, 1),
        "b2": np.asarray(b2, np.float32).reshape(PER_HOP, 1),
        "bout": np.asarray(bout, np.float32).reshape(OUT, 1),
    }
    in_maps = []
    for c in range(NCORE):
        m = dict(common)
        m.update(pc[c])
        sl = np.concatenate([
            x_tab[CH_BASE[ch] + c * ns * WIN:
                  CH_BASE[ch] + (c + 1) * ns * WIN]
            for ch, ns in enumerate(CH_SLOTS)])
        m["x_ownT"] = np.ascontiguousarray(sl.T)
        m["x_own"] = np.ascontiguousarray(sl)
        in_maps.append(m)
    return in_maps


def kernel(x, edge_index, W0, b0, W1, b1, W2, b2, Wout, bout):
    from concourse.bass_utils import run_bass_kernel_spmd

    NBL, NBH, pc, pos, posc = _preprocess(np.asarray(edge_index))
    nc = _get_nc(NBL, NBH)
    in_maps = make_in_maps(x, pc, pos, W0, b0, W1, b1, W2, b2, Wout, bout)
    res = run_bass_kernel_spmd(nc, in_maps, core_ids=list(range(NCORE)))
    big = np.concatenate([res.results[c]["out_t"] for c in range(NCORE)],
                         axis=1)
    return np.ascontiguousarray(big[:, posc].T.astype(np.float32))
